# revision 5
# baseline (speedup 1.0000x reference)
"""MixHopConv (P=(0,1,2)) Trainium2 kernel, 8-way node-sharded.

Decomposition (validated vs reference in fp64):
    deg   = in-degree clamped to >= 1;  u_0-gather operand = feat
    hop0:  agg = sum_e invsqrt(deg[src]) * feat[src]  (weighted one-hot matmul)
    u_1   = agg / deg
    hop j: u_{j+1} = (A @ u_j) / deg                  (plain one-hot matmul)
    out   = sqrt(deg) * (sum_j u_{j+1} @ U_j^T) + c
    U_j   = Wfc[:, j*D:(j+1)*D] @ W_j,   c = bfc + sum_j Wfc_j @ b_j

Sharding: nodes (and their incoming edges, bucketed by dst) are assigned to
8 cores x 49 blocks of 128 node slots, balanced by in-degree so the SPMD
program structure (chunks per (block, src-half)) is identical on every core.
Each hop gathers source rows with SWDGE dma_gather (int16 indices =>
source buffers are addressed per lo/hi half), aggregates per 128-node dst
block via one-hot matmuls accumulating in PSUM, rescales by 1/deg, and
all-gathers the new u shard across the 8 cores between hops.
"""

import math
import os
import sys

import numpy as np

for _p in ("/opt/trn_rl_repo", "/opt/pypackages"):
    if _p not in sys.path:
        sys.path.append(_p)

import concourse.bacc as bacc
import concourse.bass as bass
import concourse.mybir as mybir
import concourse.tile as tile
from concourse.bass_utils import run_bass_kernel_spmd

F32 = mybir.dt.float32
BF16 = mybir.dt.bfloat16
I16 = mybir.dt.int16

# set to BF16 to halve gather traffic + 4x matmul rate for hops 1/2
U_DTYPE = F32

LAST_RESULTS = {}  # stash for test harness (exec_time etc.)


class Cfg:
    def __init__(self, n, e, d=128, ncores=8, blk=128, sg=4):
        self.N, self.E, self.D = n, e, d
        self.NCORES, self.BLK, self.SG = ncores, blk, sg
        self.HN = n // 2                      # orig-id half boundary
        half_cores = ncores // 2
        self.NPC_REAL = n // ncores           # real nodes per core
        self.NBLK = math.ceil(self.NPC_REAL / blk)
        self.NPC = self.NBLK * blk            # padded slots per core
        self.HSLOT = self.NPC * half_cores    # slot half boundary
        self.NSLOT = self.NPC * ncores
        assert n % (2 * ncores) == 0


def _greedy_assign(items_w, nbins, cap):
    """Assign weighted items to bins (capacity cap) balancing total weight.
    items_w: [n] weights (processed in desc order). Returns bin index per item."""
    n = len(items_w)
    order = np.argsort(-items_w, kind="stable")
    loads = np.zeros(nbins)
    counts = np.zeros(nbins, dtype=np.int64)
    out = np.empty(n, dtype=np.int64)
    for i in order:
        masked = np.where(counts < cap, loads, np.inf)
        b = int(np.argmin(masked))
        out[i] = b
        loads[b] += items_w[i]
        counts[b] += 1
    return out


def _greedy_assign2(lo_w, hi_w, nbins, cap):
    """2-D balance: minimize max over bins of each dim. Returns bin per item."""
    n = len(lo_w)
    tot = lo_w + hi_w
    order = np.argsort(-tot, kind="stable")
    lo_s = np.zeros(nbins)
    hi_s = np.zeros(nbins)
    counts = np.zeros(nbins, dtype=np.int64)
    out = np.empty(n, dtype=np.int64)
    lo_t = max(lo_w.sum() / nbins, 1e-9)
    hi_t = max(hi_w.sum() / nbins, 1e-9)
    for i in order:
        score = np.maximum((lo_s + lo_w[i]) / lo_t, (hi_s + hi_w[i]) / hi_t)
        score = np.where(counts < cap, score, np.inf)
        b = int(np.argmin(score))
        out[i] = b
        lo_s[b] += lo_w[i]
        hi_s[b] += hi_w[i]
        counts[b] += 1
    return out


def _host_prep(cfg, src, dst):
    """Shard + sort edges; build all per-core device arrays and the shared
    program structure."""
    N, E, D = cfg.N, cfg.E, cfg.D
    deg = np.bincount(dst, minlength=N).astype(np.int64)
    lo_deg = np.bincount(dst[src < cfg.HN], minlength=N).astype(np.int64)
    hi_deg = deg - lo_deg
    cdeg = np.maximum(deg, 1).astype(np.float64)
    inv_sqrt = cdeg ** -0.5
    inv_deg = 1.0 / cdeg
    sqrt_deg = cdeg ** 0.5

    half_cores = cfg.NCORES // 2
    # node -> core (each half of orig ids to its own core group)
    core_of = np.empty(N, dtype=np.int64)
    for h in range(2):
        ids = np.arange(cfg.HN) + h * cfg.HN
        a = _greedy_assign(deg[ids].astype(np.float64), half_cores, cfg.NPC_REAL)
        core_of[ids] = a + h * half_cores

    # node -> (block, slot-in-block), per core
    slot_of = np.full(N, -1, dtype=np.int64)
    node_of_slot = np.full(cfg.NSLOT, -1, dtype=np.int64)
    for c in range(cfg.NCORES):
        ids = np.where(core_of == c)[0]
        blk_of = _greedy_assign2(
            lo_deg[ids].astype(np.float64), hi_deg[ids].astype(np.float64),
            cfg.NBLK, cfg.BLK)
        for b in range(cfg.NBLK):
            bids = ids[blk_of == b]
            s0 = c * cfg.NPC + b * cfg.BLK
            slots = s0 + np.arange(len(bids))
            slot_of[bids] = slots
            node_of_slot[slots] = bids

    # bucket edges by (core, block, half); count chunks
    e_core = core_of[dst]
    e_blk = (slot_of[dst] % cfg.NPC) // cfg.BLK
    e_half = (src >= cfg.HN).astype(np.int64)
    counts = np.zeros((cfg.NCORES, cfg.NBLK, 2), dtype=np.int64)
    np.add.at(counts, (e_core, e_blk, e_half), 1)
    cmax = counts.max(axis=0)  # [NBLK, 2]
    C = np.maximum(1, np.ceil(cmax / cfg.BLK).astype(np.int64))  # chunks per (blk, half)

    # shared program structure: supergroups of SG blocks
    nsg = math.ceil(cfg.NBLK / cfg.SG)
    sgs = []       # list of (lo_chunklist, hi_chunklist); chunk = (g, b, start, stop)
    g = 0
    for s in range(nsg):
        blks = list(range(s * cfg.SG, min((s + 1) * cfg.SG, cfg.NBLK)))
        calls = []
        for h in range(2):
            cl = []
            for b in blks:
                for ci in range(C[b, h]):
                    first = (h == 0 and ci == 0)
                    last = (h == 1 and ci == C[b, 1] - 1)
                    cl.append((g, b, first, last))
                    g += 1
            calls.append(cl)
        sgs.append(calls)
    nchunk = g

    # per-core edge slot arrays, in stream order
    ES = nchunk * cfg.BLK
    idx0 = np.zeros((cfg.NCORES, ES), dtype=np.int64)
    idx12 = np.zeros((cfg.NCORES, ES), dtype=np.int64)
    dstloc = np.full((cfg.NCORES, ES), -1.0, dtype=np.float32)
    isrc = np.zeros((cfg.NCORES, ES), dtype=np.float32)

    # edge order grouped by (core, blk, half)
    eorder = np.lexsort((e_half, e_blk, e_core))
    bnd = {}
    key = e_core[eorder] * (cfg.NBLK * 2) + e_blk[eorder] * 2 + e_half[eorder]
    uk, starts = np.unique(key, return_index=True)
    for k, st in zip(uk, starts):
        bnd[int(k)] = st
    key_sorted = key

    def edges_of(c, b, h):
        k = c * (cfg.NBLK * 2) + b * 2 + h
        if k not in bnd:
            return np.empty(0, dtype=np.int64)
        st = bnd[k]
        en = st
        while en < len(key_sorted) and key_sorted[en] == k:
            en += 1
        return eorder[st:en]

    # faster: use searchsorted on key_sorted
    def edges_of(c, b, h):  # noqa: F811
        k = c * (cfg.NBLK * 2) + b * 2 + h
        st = np.searchsorted(key_sorted, k, side="left")
        en = np.searchsorted(key_sorted, k, side="right")
        return eorder[st:en]

    for c in range(cfg.NCORES):
        for calls in sgs:
            for h, cl in enumerate(calls):
                for (g, b, _f, _l) in cl:
                    pass  # structure only; fill below per (b,h)
        # fill per (b, h) using chunk offsets from the structure
    # chunk offset per (b, h): first global chunk idx
    chunk_of = {}
    for calls in sgs:
        for h, cl in enumerate(calls):
            for (g, b, _f, _l) in cl:
                chunk_of.setdefault((b, h), []).append(g)

    for c in range(cfg.NCORES):
        for (b, h), glist in chunk_of.items():
            ee = edges_of(c, b, h)
            cap = len(glist) * cfg.BLK
            assert len(ee) <= cap, (c, b, h, len(ee), cap)
            base = np.array(glist)  # chunk ids
            # slot positions for these edges: chunk glist[i//128], lane i%128
            pos = (np.repeat(base, cfg.BLK).reshape(-1) * cfg.BLK
                   + np.tile(np.arange(cfg.BLK), len(glist)))[:len(ee)]
            s, d_ = src[ee], dst[ee]
            idx0[c, pos] = s - h * cfg.HN
            idx12[c, pos] = slot_of[s] - h * cfg.HSLOT
            dstloc[c, pos] = (slot_of[d_] % cfg.BLK).astype(np.float32)
            isrc[c, pos] = inv_sqrt[s].astype(np.float32)

    assert idx0.min() >= 0 and idx0.max() < 32512
    assert idx12.min() >= 0 and idx12.max() < 32512

    def wrap_idx(flat):  # [ES] -> [128, ES//16] int16 (16-wrapped, 8x replicated)
        a = flat.reshape(-1, 16).T.astype(np.int16)
        return np.ascontiguousarray(np.tile(a, (8, 1)))

    # per-block norm vectors [128, NBLK] (partition = slot in block)
    invdeg_a = np.ones((cfg.NCORES, cfg.BLK, cfg.NBLK), dtype=np.float32)
    sqrtdeg_a = np.ones((cfg.NCORES, cfg.BLK, cfg.NBLK), dtype=np.float32)
    for c in range(cfg.NCORES):
        sl = np.arange(cfg.NPC) + c * cfg.NPC
        nd = node_of_slot[sl]
        ok = nd >= 0
        iv = np.ones(cfg.NPC, dtype=np.float32)
        sq = np.ones(cfg.NPC, dtype=np.float32)
        iv[ok] = inv_deg[nd[ok]].astype(np.float32)
        sq[ok] = sqrt_deg[nd[ok]].astype(np.float32)
        invdeg_a[c] = iv.reshape(cfg.NBLK, cfg.BLK).T
        sqrtdeg_a[c] = sq.reshape(cfg.NBLK, cfg.BLK).T

    per_core = []
    for c in range(cfg.NCORES):
        per_core.append(dict(
            idx0=wrap_idx(idx0[c]),
            idx12=wrap_idx(idx12[c]),
            dstloc=np.ascontiguousarray(
                dstloc[c].reshape(nchunk, cfg.BLK).T),   # [128, nchunk]
            isrc=np.ascontiguousarray(
                isrc[c].reshape(nchunk, cfg.BLK).T),     # [128, nchunk]
            invdeg=np.ascontiguousarray(invdeg_a[c]),
            sqrtdeg=np.ascontiguousarray(sqrtdeg_a[c]),
        ))
    return dict(sgs=sgs, nchunk=nchunk, per_core=per_core,
                node_of_slot=node_of_slot, slot_of=slot_of)


def _build_program(cfg, sgs, nchunk):
    N, D = cfg.N, cfg.D
    nc = bacc.Bacc("TRN2", target_bir_lowering=False, num_devices=cfg.NCORES)

    feat_t = nc.dram_tensor("feat", [N, D], F32, kind="ExternalInput")
    idx0_t = nc.dram_tensor("idx0", [128, nchunk * 8], I16, kind="ExternalInput")
    idx12_t = nc.dram_tensor("idx12", [128, nchunk * 8], I16, kind="ExternalInput")
    dstloc_t = nc.dram_tensor("dstloc", [128, nchunk], F32, kind="ExternalInput")
    isrc_t = nc.dram_tensor("isrc", [128, nchunk], F32, kind="ExternalInput")
    invdeg_t = nc.dram_tensor("invdeg", [128, cfg.NBLK], F32, kind="ExternalInput")
    sqrtdeg_t = nc.dram_tensor("sqrtdeg", [128, cfg.NBLK], F32, kind="ExternalInput")
    iota_t = nc.dram_tensor("iota", [128, 128], F32, kind="ExternalInput")
    ident_t = nc.dram_tensor("ident", [128, 128], F32, kind="ExternalInput")
    ujt_t = nc.dram_tensor("ujt", [128, 3 * D], F32, kind="ExternalInput")
    cb_t = nc.dram_tensor("cb", [128, D], F32, kind="ExternalInput")
    out_t = nc.dram_tensor("out", [cfg.NPC, D], F32, kind="ExternalOutput")

    ushard = [nc.dram_tensor(f"ushard{j}", [cfg.NPC, D], U_DTYPE) for j in (0, 1)]
    ufull = [nc.dram_tensor(f"ufull{j}", [cfg.NSLOT, D], U_DTYPE) for j in (0, 1)]

    rg = [list(range(cfg.NCORES))]

    with tile.TileContext(nc) as tc:
        with (
            tc.tile_pool(name="const", bufs=1) as cpool,
            tc.tile_pool(name="xb", bufs=3) as xpool,
            tc.tile_pool(name="oh", bufs=6) as ohpool,
            tc.tile_pool(name="ps", bufs=4, space="PSUM") as pspool,
            tc.tile_pool(name="pt", bufs=2, space="PSUM") as ptpool,
            tc.tile_pool(name="pp", bufs=2, space="PSUM") as pppool,
            tc.tile_pool(name="ub", bufs=3) as upool,
            tc.tile_pool(name="ut", bufs=3) as utpool,
            tc.tile_pool(name="fin", bufs=3) as fpool,
        ):
            def cload(tt, shape, dtype=F32):
                s = cpool.tile(shape, dtype, tag=tt.name)
                nc.sync.dma_start(s[:], tt[:])
                return s

            iota_s = cload(iota_t, [128, 128])
            ident_s = cload(ident_t, [128, 128])
            ujt_s = cload(ujt_t, [128, 3 * D])
            cb_s = cload(cb_t, [128, D])
            invdeg_s = cload(invdeg_t, [128, cfg.NBLK])
            sqrtdeg_s = cload(sqrtdeg_t, [128, cfg.NBLK])
            dstloc_s = cload(dstloc_t, [128, nchunk])
            isrc_s = cload(isrc_t, [128, nchunk])
            idx0_s = cload(idx0_t, [128, nchunk * 8], I16)
            idx12_s = cload(idx12_t, [128, nchunk * 8], I16)
            acc = cpool.tile([128, cfg.NPC], F32, tag="acc")

            for hop in range(3):
                if hop == 0:
                    spaces = (feat_t, cfg.HN)
                    idx_s = idx0_s
                else:
                    spaces = (ufull[hop - 1], cfg.HSLOT)
                    idx_s = idx12_s
                src_t, hbound = spaces
                src_rows = src_t.shape[0]

                for calls in sgs:
                    psum_of = {}
                    for h, cl in enumerate(calls):
                        if not cl:
                            continue
                        ncol = len(cl)
                        g0 = cl[0][0]
                        nidx = ncol * cfg.BLK
                        xb = xpool.tile([128, ncol * D], U_DTYPE, tag="xb")
                        in_ap = src_t[h * hbound: h * hbound
                                      + (src_rows - hbound if h else hbound), :]
                        nc.gpsimd.dma_gather(
                            xb[:].rearrange("p (c f) -> p c f", f=D),
                            in_ap,
                            idx_s[:, g0 * 8: (g0 + ncol) * 8],
                            nidx, nidx, D,
                            single_packet=False,
                        )
                        for j, (g, b, first, last) in enumerate(cl):
                            if b not in psum_of:
                                psum_of[b] = pspool.tile([128, 128], F32, tag="ps", name="ps")
                            oh = ohpool.tile([128, 128], U_DTYPE, tag="oh")
                            if hop == 0:
                                nc.vector.tensor_scalar(
                                    oh[:], iota_s[:],
                                    dstloc_s[:, g:g + 1], isrc_s[:, g:g + 1],
                                    mybir.AluOpType.is_equal, mybir.AluOpType.mult)
                            else:
                                nc.vector.tensor_scalar(
                                    oh[:], iota_s[:],
                                    dstloc_s[:, g:g + 1], None,
                                    mybir.AluOpType.is_equal)
                            nc.tensor.matmul(
                                psum_of[b][:], oh[:], xb[:, j * D:(j + 1) * D],
                                start=first, stop=last)

                    for b in sorted(psum_of):
                        ps = psum_of[b]
                        u_sb = upool.tile([128, 128], U_DTYPE, tag="ub")
                        # u = psum / deg  (ACT: copy with per-partition scale)
                        nc.scalar.activation(
                            u_sb[:], ps[:], mybir.ActivationFunctionType.Copy,
                            scale=invdeg_s[:, b:b + 1])
                        if hop < 2:
                            nc.sync.dma_start(
                                ushard[hop][b * cfg.BLK:(b + 1) * cfg.BLK, :],
                                u_sb[:])
                        # projection: acc[:, b] += u @ U_hop^T
                        pt = ptpool.tile([128, 128], F32, tag="pt")
                        nc.tensor.transpose(pt[:], u_sb[:], ident_s[:])
                        ut = utpool.tile([128, 128], U_DTYPE, tag="ut")
                        nc.scalar.copy(ut[:], pt[:])
                        pp = pppool.tile([128, 128], F32, tag="pp")
                        nc.tensor.matmul(
                            pp[:], ut[:], ujt_s[:, hop * D:(hop + 1) * D],
                            start=True, stop=True)
                        aslice = acc[:, b * cfg.BLK:(b + 1) * cfg.BLK]
                        if hop == 0:
                            nc.vector.tensor_copy(aslice, pp[:])
                        else:
                            nc.vector.tensor_tensor(
                                aslice, aslice, pp[:], mybir.AluOpType.add)

                if hop < 2:
                    nc.gpsimd.collective_compute(
                        "AllGather", mybir.AluOpType.bypass,
                        replica_groups=rg,
                        ins=[ushard[hop].ap().opt()],
                        outs=[ufull[hop].ap().opt()],
                    )

            for b in range(cfg.NBLK):
                o1 = fpool.tile([128, D], F32, tag="fin")
                nc.scalar.activation(
                    o1[:], acc[:, b * cfg.BLK:(b + 1) * cfg.BLK],
                    mybir.ActivationFunctionType.Copy,
                    scale=sqrtdeg_s[:, b:b + 1])
                o2 = fpool.tile([128, D], F32, tag="fin2")
                nc.vector.tensor_tensor(o2[:], o1[:], cb_s[:],
                                        mybir.AluOpType.add)
                nc.sync.dma_start(out_t[b * cfg.BLK:(b + 1) * cfg.BLK, :], o2[:])

    nc.compile()
    return nc


def _prepare_weights(cfg, W, b, Wfc, bfc):
    D = cfg.D
    U = [Wfc[:, j * D:(j + 1) * D].astype(np.float64) @ W[j].astype(np.float64)
         for j in range(3)]
    c = bfc.astype(np.float64) + sum(
        Wfc[:, j * D:(j + 1) * D].astype(np.float64) @ b[j].astype(np.float64)
        for j in range(3))
    ujt = np.concatenate([u.T for u in U], axis=1).astype(np.float32)  # [D, 3D]
    cb = np.tile(c.astype(np.float32)[None, :], (128, 1))
    return np.ascontiguousarray(ujt), np.ascontiguousarray(cb)


def run(cfg, feat, W, b, Wfc, bfc, edge_index, trace=False, backend="hw"):
    src = np.asarray(edge_index[0]).astype(np.int64)
    dst = np.asarray(edge_index[1]).astype(np.int64)
    prep = _host_prep(cfg, src, dst)
    nc = _build_program(cfg, prep["sgs"], prep["nchunk"])

    ujt, cb = _prepare_weights(cfg, W, b, Wfc, bfc)
    iota = np.tile(np.arange(128, dtype=np.float32)[None, :], (128, 1))
    ident = np.eye(128, dtype=np.float32)
    featf = np.ascontiguousarray(np.asarray(feat, dtype=np.float32))

    in_maps = []
    for c in range(cfg.NCORES):
        pc = prep["per_core"][c]
        in_maps.append(dict(
            feat=featf, iota=iota, ident=ident, ujt=ujt, cb=cb,
            idx0=pc["idx0"], idx12=pc["idx12"], dstloc=pc["dstloc"],
            isrc=pc["isrc"], invdeg=pc["invdeg"], sqrtdeg=pc["sqrtdeg"],
        ))

    if backend == "sim":
        from concourse import bass_interp
        sim = bass_interp.MultiCoreSim(nc, cfg.NCORES)
        for c in range(cfg.NCORES):
            for k, v in in_maps[c].items():
                sim.cores[c].tensor(k)[:] = v
        sim.simulate()
        results = [{"out": np.array(sim.cores[c].mem_tensor("out"))}
                   for c in range(cfg.NCORES)]
    else:
        res = run_bass_kernel_spmd(nc, in_maps,
                                   core_ids=list(range(cfg.NCORES)),
                                   trace=trace)
        LAST_RESULTS["bass"] = res
        results = res.results

    out = np.empty((cfg.N, cfg.D), dtype=np.float32)
    nos = prep["node_of_slot"]
    for c in range(cfg.NCORES):
        shard = results[c]["out"]
        sl = np.arange(cfg.NPC) + c * cfg.NPC
        nd = nos[sl]
        ok = nd >= 0
        out[nd[ok]] = shard[ok]
    return out


def kernel(feat, W0, b0, W1, b1, W2, b2, Wfc, bfc, edge_index):
    cfg = Cfg(50000, 625000)
    trace = bool(int(os.environ.get("KERNEL_TRACE", "0")))
    return run(cfg, feat, [W0, W1, W2], [b0, b1, b2], Wfc, bfc,
               np.asarray(edge_index), trace=trace)


# revision 10
# speedup vs baseline: 1.1618x; 1.1618x over previous
"""MixHopConv (P=(0,1,2)) Trainium2 kernel, 8-way node-sharded.

Decomposition (validated vs reference in fp64):
    deg   = in-degree clamped to >= 1;  u_0-gather operand = feat
    hop0:  agg = sum_e invsqrt(deg[src]) * feat[src]  (weighted one-hot matmul)
    u_1   = agg / deg
    hop j: u_{j+1} = (A @ u_j) / deg                  (plain one-hot matmul)
    out   = sqrt(deg) * (sum_j u_{j+1} @ U_j^T) + c
    U_j   = Wfc[:, j*D:(j+1)*D] @ W_j,   c = bfc + sum_j Wfc_j @ b_j

Sharding: nodes (and their incoming edges, bucketed by dst) are assigned to
8 cores x 49 blocks of 128 node slots, balanced by in-degree so the SPMD
program structure (chunks per (block, src-half)) is identical on every core.
Each hop gathers source rows with SWDGE dma_gather (int16 indices =>
source buffers are addressed per lo/hi half), aggregates per 128-node dst
block via one-hot matmuls accumulating in PSUM, rescales by 1/deg, and
all-gathers the new u shard across the 8 cores between hops.
"""

import math
import os
import sys

import numpy as np

for _p in ("/opt/trn_rl_repo", "/opt/pypackages"):
    if _p not in sys.path:
        sys.path.append(_p)

import concourse.bacc as bacc
import concourse.bass as bass
import concourse.mybir as mybir
import concourse.tile as tile
from concourse.bass_utils import run_bass_kernel_spmd

F32 = mybir.dt.float32
BF16 = mybir.dt.bfloat16
I16 = mybir.dt.int16

# set to BF16 to halve gather traffic + 4x matmul rate for hops 1/2
U_DTYPE = F32

LAST_RESULTS = {}  # stash for test harness (exec_time etc.)


class Cfg:
    def __init__(self, n, e, d=128, ncores=8, blk=128, sg=4):
        self.N, self.E, self.D = n, e, d
        self.NCORES, self.BLK, self.SG = ncores, blk, sg
        self.HN = n // 2                      # orig-id half boundary
        half_cores = ncores // 2
        self.NPC_REAL = n // ncores           # real nodes per core
        self.NBLK = math.ceil(self.NPC_REAL / blk)
        self.NPC = self.NBLK * blk            # padded slots per core
        self.HSLOT = self.NPC * half_cores    # slot half boundary
        self.NSLOT = self.NPC * ncores
        assert n % (2 * ncores) == 0


def _greedy_assign(items_w, nbins, cap):
    """Assign weighted items to bins (capacity cap) balancing total weight.
    items_w: [n] weights (processed in desc order). Returns bin index per item."""
    n = len(items_w)
    order = np.argsort(-items_w, kind="stable")
    loads = np.zeros(nbins)
    counts = np.zeros(nbins, dtype=np.int64)
    out = np.empty(n, dtype=np.int64)
    for i in order:
        masked = np.where(counts < cap, loads, np.inf)
        b = int(np.argmin(masked))
        out[i] = b
        loads[b] += items_w[i]
        counts[b] += 1
    return out


def _greedy_assign2(lo_w, hi_w, nbins, cap):
    """2-D balance: minimize max over bins of each dim. Returns bin per item."""
    n = len(lo_w)
    tot = lo_w + hi_w
    order = np.argsort(-tot, kind="stable")
    lo_s = np.zeros(nbins)
    hi_s = np.zeros(nbins)
    counts = np.zeros(nbins, dtype=np.int64)
    out = np.empty(n, dtype=np.int64)
    lo_t = max(lo_w.sum() / nbins, 1e-9)
    hi_t = max(hi_w.sum() / nbins, 1e-9)
    for i in order:
        score = np.maximum((lo_s + lo_w[i]) / lo_t, (hi_s + hi_w[i]) / hi_t)
        score = np.where(counts < cap, score, np.inf)
        b = int(np.argmin(score))
        out[i] = b
        lo_s[b] += lo_w[i]
        hi_s[b] += hi_w[i]
        counts[b] += 1
    return out


def _greedy_assign_target(lo_w, hi_w, cap_lo, cap_hi, cap_n):
    """Best-fit-decreasing into bins with per-bin (lo, hi) edge capacities and
    node-count cap. Returns bin per item; overflows allowed (min-overflow)."""
    n = len(lo_w)
    nbins = len(cap_lo)
    order = np.argsort(-(lo_w + hi_w), kind="stable")
    lo_s = np.zeros(nbins)
    hi_s = np.zeros(nbins)
    counts = np.zeros(nbins, dtype=np.int64)
    out = np.empty(n, dtype=np.int64)
    for i in order:
        nlo, nhi = lo_w[i], hi_w[i]
        open_ = counts < cap_n
        ov_lo = np.maximum(lo_s + nlo - cap_lo, 0.0)
        ov_hi = np.maximum(hi_s + nhi - cap_hi, 0.0)
        ov = ov_lo + ov_hi
        feas = open_ & (ov <= 0)
        if feas.any():
            # tightest fit: least remaining capacity after placement
            slack = (cap_lo - lo_s - nlo) + (cap_hi - hi_s - nhi)
            slack = np.where(feas, slack, np.inf)
            b = int(np.argmin(slack))
        else:
            ov = np.where(open_, ov, np.inf)
            b = int(np.argmin(ov))
        out[i] = b
        lo_s[b] += nlo
        hi_s[b] += nhi
        counts[b] += 1
    return out


def _chunk_targets(total_chunks, nbins, phase=0):
    """Distribute total_chunks over nbins as evenly as possible; the +1
    remainder bins start at `phase` (so lo/hi remainders land on different
    blocks)."""
    base = total_chunks // nbins
    rem = total_chunks % nbins
    t = np.full(nbins, base, dtype=np.int64)
    for k in range(rem):
        t[(phase + k) % nbins] += 1
    return t


def _host_prep(cfg, src, dst):
    """Shard + sort edges; build all per-core device arrays and the shared
    program structure."""
    N, E, D = cfg.N, cfg.E, cfg.D
    deg = np.bincount(dst, minlength=N).astype(np.int64)
    lo_deg = np.bincount(dst[src < cfg.HN], minlength=N).astype(np.int64)
    hi_deg = deg - lo_deg
    cdeg = np.maximum(deg, 1).astype(np.float64)
    inv_sqrt = cdeg ** -0.5
    inv_deg = 1.0 / cdeg
    sqrt_deg = cdeg ** 0.5

    half_cores = cfg.NCORES // 2
    # node -> core (each half of orig ids to its own core group), balancing
    # lo- and hi-sourced in-edges jointly
    core_of = np.empty(N, dtype=np.int64)
    for h in range(2):
        ids = np.arange(cfg.HN) + h * cfg.HN
        a = _greedy_assign2(lo_deg[ids].astype(np.float64),
                            hi_deg[ids].astype(np.float64),
                            half_cores, cfg.NPC_REAL)
        core_of[ids] = a + h * half_cores

    # per-block chunk targets (identical across cores, SPMD requirement)
    lo_tot = np.array([lo_deg[core_of == c].sum() for c in range(cfg.NCORES)])
    hi_tot = np.array([hi_deg[core_of == c].sum() for c in range(cfg.NCORES)])
    t_lo = _chunk_targets(int(math.ceil(lo_tot.max() / cfg.BLK)), cfg.NBLK, 0)
    t_hi = _chunk_targets(int(math.ceil(hi_tot.max() / cfg.BLK)),
                          cfg.NBLK, cfg.NBLK // 2)

    # node -> (block, slot-in-block), per core: fit under the chunk targets
    slot_of = np.full(N, -1, dtype=np.int64)
    node_of_slot = np.full(cfg.NSLOT, -1, dtype=np.int64)
    for c in range(cfg.NCORES):
        ids = np.where(core_of == c)[0]
        blk_of = _greedy_assign_target(
            lo_deg[ids].astype(np.float64), hi_deg[ids].astype(np.float64),
            (t_lo * cfg.BLK).astype(np.float64),
            (t_hi * cfg.BLK).astype(np.float64), cfg.BLK)
        for b in range(cfg.NBLK):
            bids = ids[blk_of == b]
            s0 = c * cfg.NPC + b * cfg.BLK
            slots = s0 + np.arange(len(bids))
            slot_of[bids] = slots
            node_of_slot[slots] = bids

    # bucket edges by (core, block, half); count chunks
    e_core = core_of[dst]
    e_blk = (slot_of[dst] % cfg.NPC) // cfg.BLK
    e_half = (src >= cfg.HN).astype(np.int64)
    counts = np.zeros((cfg.NCORES, cfg.NBLK, 2), dtype=np.int64)
    np.add.at(counts, (e_core, e_blk, e_half), 1)
    cmax = counts.max(axis=0)  # [NBLK, 2]
    C = np.maximum(1, np.ceil(cmax / cfg.BLK).astype(np.int64))  # chunks per (blk, half)

    # shared program structure: supergroups of SG blocks
    nsg = math.ceil(cfg.NBLK / cfg.SG)
    sgs = []       # list of (lo_chunklist, hi_chunklist); chunk = (g, b, start, stop)
    g = 0
    for s in range(nsg):
        blks = list(range(s * cfg.SG, min((s + 1) * cfg.SG, cfg.NBLK)))
        calls = []
        for h in range(2):
            cl = []
            for b in blks:
                for ci in range(C[b, h]):
                    first = (h == 0 and ci == 0)
                    last = (h == 1 and ci == C[b, 1] - 1)
                    cl.append((g, b, first, last))
                    g += 1
            calls.append(cl)
        sgs.append(calls)
    nchunk = g

    # per-core edge slot arrays, in stream order
    ES = nchunk * cfg.BLK
    idx0 = np.zeros((cfg.NCORES, ES), dtype=np.int64)
    idx12 = np.zeros((cfg.NCORES, ES), dtype=np.int64)
    dstloc = np.full((cfg.NCORES, ES), -1.0, dtype=np.float32)
    isrc = np.zeros((cfg.NCORES, ES), dtype=np.float32)

    # edge order grouped by (core, blk, half)
    eorder = np.lexsort((e_half, e_blk, e_core))
    bnd = {}
    key = e_core[eorder] * (cfg.NBLK * 2) + e_blk[eorder] * 2 + e_half[eorder]
    uk, starts = np.unique(key, return_index=True)
    for k, st in zip(uk, starts):
        bnd[int(k)] = st
    key_sorted = key

    def edges_of(c, b, h):
        k = c * (cfg.NBLK * 2) + b * 2 + h
        if k not in bnd:
            return np.empty(0, dtype=np.int64)
        st = bnd[k]
        en = st
        while en < len(key_sorted) and key_sorted[en] == k:
            en += 1
        return eorder[st:en]

    # faster: use searchsorted on key_sorted
    def edges_of(c, b, h):  # noqa: F811
        k = c * (cfg.NBLK * 2) + b * 2 + h
        st = np.searchsorted(key_sorted, k, side="left")
        en = np.searchsorted(key_sorted, k, side="right")
        return eorder[st:en]

    for c in range(cfg.NCORES):
        for calls in sgs:
            for h, cl in enumerate(calls):
                for (g, b, _f, _l) in cl:
                    pass  # structure only; fill below per (b,h)
        # fill per (b, h) using chunk offsets from the structure
    # chunk offset per (b, h): first global chunk idx
    chunk_of = {}
    for calls in sgs:
        for h, cl in enumerate(calls):
            for (g, b, _f, _l) in cl:
                chunk_of.setdefault((b, h), []).append(g)

    for c in range(cfg.NCORES):
        for (b, h), glist in chunk_of.items():
            ee = edges_of(c, b, h)
            cap = len(glist) * cfg.BLK
            assert len(ee) <= cap, (c, b, h, len(ee), cap)
            base = np.array(glist)  # chunk ids
            # slot positions for these edges: chunk glist[i//128], lane i%128
            pos = (np.repeat(base, cfg.BLK).reshape(-1) * cfg.BLK
                   + np.tile(np.arange(cfg.BLK), len(glist)))[:len(ee)]
            s, d_ = src[ee], dst[ee]
            idx0[c, pos] = s - h * cfg.HN
            idx12[c, pos] = slot_of[s] - h * cfg.HSLOT
            dstloc[c, pos] = (slot_of[d_] % cfg.BLK).astype(np.float32)
            isrc[c, pos] = inv_sqrt[s].astype(np.float32)

    assert idx0.min() >= 0 and idx0.max() < 32512
    assert idx12.min() >= 0 and idx12.max() < 32512

    def wrap_idx(flat):  # [ES] -> [128, ES//16] int16 (16-wrapped, 8x replicated)
        a = flat.reshape(-1, 16).T.astype(np.int16)
        return np.ascontiguousarray(np.tile(a, (8, 1)))

    # per-block norm vectors [128, NBLK] (partition = slot in block)
    invdeg_a = np.ones((cfg.NCORES, cfg.BLK, cfg.NBLK), dtype=np.float32)
    sqrtdeg_a = np.ones((cfg.NCORES, cfg.BLK, cfg.NBLK), dtype=np.float32)
    for c in range(cfg.NCORES):
        sl = np.arange(cfg.NPC) + c * cfg.NPC
        nd = node_of_slot[sl]
        ok = nd >= 0
        iv = np.ones(cfg.NPC, dtype=np.float32)
        sq = np.ones(cfg.NPC, dtype=np.float32)
        iv[ok] = inv_deg[nd[ok]].astype(np.float32)
        sq[ok] = sqrt_deg[nd[ok]].astype(np.float32)
        invdeg_a[c] = iv.reshape(cfg.NBLK, cfg.BLK).T
        sqrtdeg_a[c] = sq.reshape(cfg.NBLK, cfg.BLK).T

    per_core = []
    for c in range(cfg.NCORES):
        per_core.append(dict(
            idx0=wrap_idx(idx0[c]),
            idx12=wrap_idx(idx12[c]),
            dstloc=np.ascontiguousarray(
                dstloc[c].reshape(nchunk, cfg.BLK).T),   # [128, nchunk]
            isrc=np.ascontiguousarray(
                isrc[c].reshape(nchunk, cfg.BLK).T),     # [128, nchunk]
            invdeg=np.ascontiguousarray(invdeg_a[c]),
            sqrtdeg=np.ascontiguousarray(sqrtdeg_a[c]),
        ))
    return dict(sgs=sgs, nchunk=nchunk, per_core=per_core,
                node_of_slot=node_of_slot, slot_of=slot_of)


def _build_program(cfg, sgs, nchunk):
    N, D = cfg.N, cfg.D
    nc = bacc.Bacc("TRN2", target_bir_lowering=False, num_devices=cfg.NCORES,
                   num_swdge_queues=4)

    feat_t = nc.dram_tensor("feat", [N, D], F32, kind="ExternalInput")
    idx0_t = nc.dram_tensor("idx0", [128, nchunk * 8], I16, kind="ExternalInput")
    idx12_t = nc.dram_tensor("idx12", [128, nchunk * 8], I16, kind="ExternalInput")
    dstloc_t = nc.dram_tensor("dstloc", [128, nchunk], F32, kind="ExternalInput")
    isrc_t = nc.dram_tensor("isrc", [128, nchunk], F32, kind="ExternalInput")
    invdeg_t = nc.dram_tensor("invdeg", [128, cfg.NBLK], F32, kind="ExternalInput")
    sqrtdeg_t = nc.dram_tensor("sqrtdeg", [128, cfg.NBLK], F32, kind="ExternalInput")
    iota_t = nc.dram_tensor("iota", [128, 128], F32, kind="ExternalInput")
    ident_t = nc.dram_tensor("ident", [128, 128], F32, kind="ExternalInput")
    ujt_t = nc.dram_tensor("ujt", [128, 3 * D], F32, kind="ExternalInput")
    cb_t = nc.dram_tensor("cb", [128, D], F32, kind="ExternalInput")
    out_t = nc.dram_tensor("out", [cfg.NPC, D], F32, kind="ExternalOutput")

    ushard = [nc.dram_tensor(f"ushard{j}", [cfg.NPC, D], U_DTYPE) for j in (0, 1)]
    ufull = [nc.dram_tensor(f"ufull{j}", [cfg.NSLOT, D], U_DTYPE) for j in (0, 1)]

    rg = [list(range(cfg.NCORES))]

    with tile.TileContext(nc) as tc:
        with (
            tc.tile_pool(name="const", bufs=1) as cpool,
            tc.tile_pool(name="xb", bufs=4) as xpool,
            tc.tile_pool(name="oh", bufs=6) as ohpool,
            tc.tile_pool(name="ps", bufs=6, space="PSUM") as pspool,
            tc.tile_pool(name="pt", bufs=1, space="PSUM") as ptpool,
            tc.tile_pool(name="pp", bufs=1, space="PSUM") as pppool,
            tc.tile_pool(name="ub", bufs=3) as upool,
            tc.tile_pool(name="ut", bufs=3) as utpool,
            tc.tile_pool(name="fin", bufs=3) as fpool,
        ):
            def cload(tt, shape, dtype=F32):
                s = cpool.tile(shape, dtype, tag=tt.name)
                nc.sync.dma_start(s[:], tt[:])
                return s

            iota_s = cload(iota_t, [128, 128])
            ident_s = cload(ident_t, [128, 128])
            ujt_s = cload(ujt_t, [128, 3 * D])
            cb_s = cload(cb_t, [128, D])
            invdeg_s = cload(invdeg_t, [128, cfg.NBLK])
            sqrtdeg_s = cload(sqrtdeg_t, [128, cfg.NBLK])
            dstloc_s = cload(dstloc_t, [128, nchunk])
            isrc_s = cload(isrc_t, [128, nchunk])
            idx0_s = cload(idx0_t, [128, nchunk * 8], I16)
            idx12_s = cload(idx12_t, [128, nchunk * 8], I16)
            acc = cpool.tile([128, cfg.NPC], F32, tag="acc")

            for hop in range(3):
                if hop == 0:
                    spaces = (feat_t, cfg.HN)
                    idx_s = idx0_s
                else:
                    spaces = (ufull[hop - 1], cfg.HSLOT)
                    idx_s = idx12_s
                src_t, hbound = spaces
                src_rows = src_t.shape[0]

                call_i = 0
                for calls in sgs:
                    psum_of = {}
                    for h, cl in enumerate(calls):
                        if not cl:
                            continue
                        # split into ring-safe segments (<=16 chunks = 2048
                        # descriptors per SWDGE call), striped over queues
                        for seg0 in range(0, len(cl), 16):
                            seg = cl[seg0:seg0 + 16]
                            ncol = len(seg)
                            g0 = seg[0][0]
                            nidx = ncol * cfg.BLK
                            xb = xpool.tile([128, ncol * D], U_DTYPE, tag="xb")
                            in_ap = src_t[h * hbound: h * hbound
                                          + (src_rows - hbound if h else hbound), :]
                            nc.gpsimd.dma_gather(
                                xb[:].rearrange("p (c f) -> p c f", f=D),
                                in_ap,
                                idx_s[:, g0 * 8: (g0 + ncol) * 8],
                                nidx, nidx, D,
                                single_packet=False,
                                queue_num=(hop + call_i) % 4,
                            )
                            call_i += 1
                            for j, (g, b, first, last) in enumerate(seg):
                                if b not in psum_of:
                                    psum_of[b] = pspool.tile(
                                        [128, 128], F32, tag="ps", name="ps")
                                oh = ohpool.tile([128, 128], U_DTYPE, tag="oh")
                                if hop == 0:
                                    nc.vector.tensor_scalar(
                                        oh[:], iota_s[:],
                                        dstloc_s[:, g:g + 1], isrc_s[:, g:g + 1],
                                        mybir.AluOpType.is_equal,
                                        mybir.AluOpType.mult)
                                else:
                                    nc.vector.tensor_scalar(
                                        oh[:], iota_s[:],
                                        dstloc_s[:, g:g + 1], None,
                                        mybir.AluOpType.is_equal)
                                nc.tensor.matmul(
                                    psum_of[b][:], oh[:],
                                    xb[:, j * D:(j + 1) * D],
                                    start=first, stop=last)

                    for b in sorted(psum_of):
                        ps = psum_of[b]
                        u_sb = upool.tile([128, 128], U_DTYPE, tag="ub")
                        # u = psum / deg  (ACT: copy with per-partition scale)
                        nc.scalar.activation(
                            u_sb[:], ps[:], mybir.ActivationFunctionType.Copy,
                            scale=invdeg_s[:, b:b + 1])
                        if hop < 2:
                            nc.sync.dma_start(
                                ushard[hop][b * cfg.BLK:(b + 1) * cfg.BLK, :],
                                u_sb[:])
                        # projection: acc[:, b] += u @ U_hop^T
                        pt = ptpool.tile([128, 128], F32, tag="pt")
                        nc.tensor.transpose(pt[:], u_sb[:], ident_s[:])
                        ut = utpool.tile([128, 128], U_DTYPE, tag="ut")
                        nc.scalar.copy(ut[:], pt[:])
                        pp = pppool.tile([128, 128], F32, tag="pp")
                        nc.tensor.matmul(
                            pp[:], ut[:], ujt_s[:, hop * D:(hop + 1) * D],
                            start=True, stop=True)
                        aslice = acc[:, b * cfg.BLK:(b + 1) * cfg.BLK]
                        if hop == 0:
                            nc.vector.tensor_copy(aslice, pp[:])
                        else:
                            nc.vector.tensor_tensor(
                                aslice, aslice, pp[:], mybir.AluOpType.add)

                if hop < 2:
                    nc.gpsimd.collective_compute(
                        "AllGather", mybir.AluOpType.bypass,
                        replica_groups=rg,
                        ins=[ushard[hop].ap().opt()],
                        outs=[ufull[hop].ap().opt()],
                    )

            for b in range(cfg.NBLK):
                o1 = fpool.tile([128, D], F32, tag="fin")
                nc.scalar.activation(
                    o1[:], acc[:, b * cfg.BLK:(b + 1) * cfg.BLK],
                    mybir.ActivationFunctionType.Copy,
                    scale=sqrtdeg_s[:, b:b + 1])
                o2 = fpool.tile([128, D], F32, tag="fin2")
                nc.vector.tensor_tensor(o2[:], o1[:], cb_s[:],
                                        mybir.AluOpType.add)
                nc.sync.dma_start(out_t[b * cfg.BLK:(b + 1) * cfg.BLK, :], o2[:])

    nc.compile()
    return nc


def _prepare_weights(cfg, W, b, Wfc, bfc):
    D = cfg.D
    U = [Wfc[:, j * D:(j + 1) * D].astype(np.float64) @ W[j].astype(np.float64)
         for j in range(3)]
    c = bfc.astype(np.float64) + sum(
        Wfc[:, j * D:(j + 1) * D].astype(np.float64) @ b[j].astype(np.float64)
        for j in range(3))
    ujt = np.concatenate([u.T for u in U], axis=1).astype(np.float32)  # [D, 3D]
    cb = np.tile(c.astype(np.float32)[None, :], (128, 1))
    return np.ascontiguousarray(ujt), np.ascontiguousarray(cb)


def run(cfg, feat, W, b, Wfc, bfc, edge_index, trace=False, backend="hw"):
    src = np.asarray(edge_index[0]).astype(np.int64)
    dst = np.asarray(edge_index[1]).astype(np.int64)
    prep = _host_prep(cfg, src, dst)
    nc = _build_program(cfg, prep["sgs"], prep["nchunk"])

    ujt, cb = _prepare_weights(cfg, W, b, Wfc, bfc)
    iota = np.tile(np.arange(128, dtype=np.float32)[None, :], (128, 1))
    ident = np.eye(128, dtype=np.float32)
    featf = np.ascontiguousarray(np.asarray(feat, dtype=np.float32))

    in_maps = []
    for c in range(cfg.NCORES):
        pc = prep["per_core"][c]
        in_maps.append(dict(
            feat=featf, iota=iota, ident=ident, ujt=ujt, cb=cb,
            idx0=pc["idx0"], idx12=pc["idx12"], dstloc=pc["dstloc"],
            isrc=pc["isrc"], invdeg=pc["invdeg"], sqrtdeg=pc["sqrtdeg"],
        ))

    if backend == "sim":
        from concourse import bass_interp
        sim = bass_interp.MultiCoreSim(nc, cfg.NCORES)
        for c in range(cfg.NCORES):
            for k, v in in_maps[c].items():
                sim.cores[c].tensor(k)[:] = v
        sim.simulate()
        results = [{"out": np.array(sim.cores[c].mem_tensor("out"))}
                   for c in range(cfg.NCORES)]
    else:
        res = run_bass_kernel_spmd(nc, in_maps,
                                   core_ids=list(range(cfg.NCORES)),
                                   trace=trace)
        LAST_RESULTS["bass"] = res
        results = res.results

    out = np.empty((cfg.N, cfg.D), dtype=np.float32)
    nos = prep["node_of_slot"]
    for c in range(cfg.NCORES):
        shard = results[c]["out"]
        sl = np.arange(cfg.NPC) + c * cfg.NPC
        nd = nos[sl]
        ok = nd >= 0
        out[nd[ok]] = shard[ok]
    return out


def kernel(feat, W0, b0, W1, b1, W2, b2, Wfc, bfc, edge_index):
    cfg = Cfg(50000, 625000)
    trace = bool(int(os.environ.get("KERNEL_TRACE", "0")))
    return run(cfg, feat, [W0, W1, W2], [b0, b1, b2], Wfc, bfc,
               np.asarray(edge_index), trace=trace)
